# revision 32
# baseline (speedup 1.0000x reference)
"""Trainium2 Bass kernel for nn_Attention_Critic (gnn_message_passing).

Strategy v3: data-parallel over the batch (8 cores x 4096), feature-major
layout on chip ([features, batch]), BatchNorm folded into first-layer
weights ON HOST (stats over the full batch in numpy -- no device stats,
no AllReduce), attention-weight products folded on host (sel@key^T),
attention dots via cheap 1-column PE reduce matmuls into a batch-major
[128, 64] logit tile (slot-major columns 8p+u), softmax with tiny
batch-major vector ops, weights transposed to row-major via a single PE
transpose, stored to DRAM as contiguous [slots, 1024] rows and broadcast
back via stride-0 row reads.  The per-pair schedule is phased
(A: encode+dots+softmax for all agents; B1: broadcasts+weighted sums;
B2: merge matmuls) so the PE never sits behind the broadcast round trip
and no PSUM tag is held across another agent's encode.  bf16 matmuls,
fp32 PSUM.
"""
import os
import sys

sys.path.insert(0, "/opt/trn_rl_repo")

import numpy as np
import ml_dtypes
from contextlib import ExitStack

import concourse.bass as bass
import concourse.tile as tile
from concourse import bacc, mybir
from concourse.bass_utils import run_bass_kernel_spmd

# Pin every activation to the natural_log_exp_and_others table set (covers
# Exp/Prelu/Identity/Copy) so the whole kernel needs exactly one
# ACT_TABLE_LOAD instead of thrashing between per-function sets.
_ORIG_GAT = bacc.get_activation_tables


def _pinned_tables(arch):
    t = _ORIG_GAT(arch)
    return {k: (v if k == "natural_log_exp_and_others" else set())
            for k, v in t.items()}


bacc.get_activation_tables = _pinned_tables

NA, B, H = 3, 32768, 128
EPS = 1e-5
NCORES = 8
BS = B // NCORES          # 4096 per core
NT = 512                  # batch tile
ITERS = BS // NT          # 8
NPAIR = ITERS // 2        # 4 iteration pairs
SCALE = 1.0 / np.sqrt(H)

bf16 = mybir.dt.bfloat16
f32 = mybir.dt.float32

ENT_ROWS = 208            # per-agent stride in entd (96-aligned groups)
# (name, group, tile partition base, K)
BLOCKS = [("en", "A", 0, 6), ("oa0", "A", 32, 4), ("oa1", "A", 64, 4),
          ("g0", "B", 0, 2), ("g1", "B", 32, 2), ("g2", "B", 64, 2),
          ("senc", "C", 0, 18)]
BIGW = (["wsk0", "wsk1", "aval0", "aval1", "mcrit", "cvalw"]
        + [f"m_en{n}" for n in range(NA)] + [f"m_ov0{n}" for n in range(NA)]
        + [f"m_ov1{n}" for n in range(NA)] + [f"cw1a{n}" for n in range(NA)]
        + [f"cw1b{n}" for n in range(NA)])
BIASC = ["avb0", "avb1", "mb0", "mb1", "mb2", "cvb", "cb10", "cb11", "cb12"]


def _b16(x):
    return np.asarray(x, np.float32).astype(ml_dtypes.bfloat16)


def _prep_ent_blocks(s, a, lo, hi):
    bs = hi - lo
    out = np.zeros((NA * ENT_ROWS, bs), np.float32)
    for n in range(NA):
        sn = s[n, lo:hi].T
        an = a[n, lo:hi].T
        r = ENT_ROWS * n
        out[r + 0:r + 4] = sn[0:4]
        out[r + 4:r + 6] = an[0:2]
        out[r + 6] = 1.0
        out[r + 32:r + 36] = sn[4:8]
        out[r + 36] = 1.0
        out[r + 64:r + 68] = sn[8:12]
        out[r + 68] = 1.0
        out[r + 96:r + 98] = sn[12:14]
        out[r + 98] = 1.0
        out[r + 128:r + 130] = sn[14:16]
        out[r + 130] = 1.0
        out[r + 160:r + 162] = sn[16:18]
        out[r + 162] = 1.0
        out[r + 176:r + 194] = sn[0:18]
        out[r + 194] = 1.0
    return _b16(out)


def _fold(W, b, mean, std):
    """BatchNorm fold: y = ((x-m)/std) @ W + b  ->  x @ Wf + bf."""
    Wf = W / std[:, None]
    bf = b - (mean / std) @ W
    return Wf, bf


def _prep_l1w(inp):
    s, a = inp["s"], inp["a"]
    A = np.zeros((NA * 69, 128), np.float32)
    Bm = np.zeros((NA * 67, 128), np.float32)
    C = np.zeros((NA * 19, 128), np.float32)
    for n in range(NA):
        feats = np.concatenate([s[n], a[n][:, :2]], 1).astype(np.float64)
        mean = feats.mean(0).astype(np.float32)
        std = np.sqrt(feats.var(0) + EPS).astype(np.float32)

        def fold(W, b, idx):
            return _fold(W, b, mean[idx], std[idx])

        Wf, bf = fold(inp["en_W"][n], inp["en_b"][n], [0, 1, 2, 3, 18, 19])
        A[69 * n + 0:69 * n + 6] = Wf
        A[69 * n + 6] = bf
        Wf, bf = fold(inp["oa_W"][n], inp["oa_b"][n], [4, 5, 6, 7])
        A[69 * n + 32:69 * n + 36] = Wf
        A[69 * n + 36] = bf
        Wf, bf = fold(inp["oa_W"][n], inp["oa_b"][n], [8, 9, 10, 11])
        A[69 * n + 64:69 * n + 68] = Wf
        A[69 * n + 68] = bf
        for gi, base in enumerate((0, 32, 64)):
            idx = [12 + 2 * gi, 13 + 2 * gi]
            Wf, bf = fold(inp["goal_W"][n], inp["goal_b"][n], idx)
            Bm[67 * n + base:67 * n + base + 2] = Wf
            Bm[67 * n + base + 2] = bf
        Wf, bf = fold(inp["senc_W"][n], inp["senc_b"][n], list(range(18)))
        C[19 * n + 0:19 * n + 18] = Wf
        C[19 * n + 18] = bf
    return _b16(A), _b16(Bm), _b16(C)


def _prep_bigw(inp):
    w = {}
    w["wsk0"] = inp["asel_W"][0] @ inp["akey_W"][0].T
    w["wsk1"] = inp["asel_W"][1] @ inp["akey_W"][1].T
    w["aval0"] = inp["aval_W"][0]
    w["aval1"] = inp["aval_W"][1]
    w["mcrit"] = inp["ckey_W"][0] @ inp["csel_W"][0].T
    w["cvalw"] = inp["cval_W"][0]
    for n in range(NA):
        w[f"m_en{n}"] = inp["merge_W"][n, 0:128]
        w[f"m_ov0{n}"] = inp["merge_W"][n, 128:256]
        w[f"m_ov1{n}"] = inp["merge_W"][n, 256:384]
        w[f"cw1a{n}"] = inp["cW1"][n, 0:128]
        w[f"cw1b{n}"] = inp["cW1"][n, 128:256]
    return _b16(np.concatenate([w[k] for k in BIGW], 0))


def _prep_bias(inp):
    cols = [inp["aval_b"][0], inp["aval_b"][1],
            inp["merge_b"][0], inp["merge_b"][1], inp["merge_b"][2],
            inp["cval_b"][0], inp["cb1"][0], inp["cb1"][1], inp["cb1"][2]]
    return np.stack(cols, 1).astype(np.float32)


_NC_CACHE = {}


def _build_nc():
    nc = bacc.Bacc("TRN2", target_bir_lowering=False, debug=False,
                   num_devices=NCORES)
    entd = nc.dram_tensor("entd", [NA * ENT_ROWS, BS], bf16,
                          kind="ExternalInput")
    l1Ad = nc.dram_tensor("l1Ad", [NA * 69, 128], bf16, kind="ExternalInput")
    l1Bd = nc.dram_tensor("l1Bd", [NA * 67, 128], bf16, kind="ExternalInput")
    l1Cd = nc.dram_tensor("l1Cd", [NA * 19, 128], bf16, kind="ExternalInput")
    bigwd = nc.dram_tensor("bigwd", [21 * 128, 128], bf16,
                           kind="ExternalInput")
    cw2d = nc.dram_tensor("cw2d", [NA * 128, 2], bf16, kind="ExternalInput")
    biasd = nc.dram_tensor("biasd", [128, 9], f32, kind="ExternalInput")
    cb2d = nc.dram_tensor("cb2d", [2, NA], f32, kind="ExternalInput")
    identd = nc.dram_tensor("identd", [128, 128], bf16, kind="ExternalInput")
    outd = nc.dram_tensor("outd", [6, BS], f32, kind="ExternalOutput")

    wscrd = nc.dram_tensor("wscrd", [NPAIR, 4, 8, 1024], bf16)

    with tile.TileContext(nc) as tc, ExitStack() as ctx:
        wp = ctx.enter_context(tc.tile_pool(name="wp", bufs=1))
        io = ctx.enter_context(tc.tile_pool(name="io", bufs=1))
        wk = ctx.enter_context(tc.tile_pool(name="wk", bufs=2))
        pp = ctx.enter_context(tc.tile_pool(name="pp", bufs=1, space="PSUM"))

        lwA, lwB, lwC = {}, {}, {}
        for n in range(NA):
            tA = wp.tile([69, 128], bf16, name=f"lwA{n}")
            nc.scalar.dma_start(tA[:], l1Ad[69 * n:69 * (n + 1), :])
            lwA[n] = tA
            tB = wp.tile([67, 128], bf16, name=f"lwB{n}")
            nc.scalar.dma_start(tB[:], l1Bd[67 * n:67 * (n + 1), :])
            lwB[n] = tB
            tC = wp.tile([19, 128], bf16, name=f"lwC{n}")
            nc.scalar.dma_start(tC[:], l1Cd[19 * n:19 * (n + 1), :])
            lwC[n] = tC
        ebC = {}
        for n in range(NA):
            o = ENT_ROWS * n
            t = io.tile([19, BS], bf16, name=f"ebC{n}")
            nc.sync.dma_start(t[:], entd[o + 176:o + 195, :])
            ebC[n] = t
        big = {}
        _qs = [nc.sync, nc.scalar]
        for idx, name in enumerate(BIGW):
            t = wp.tile([128, 128], bf16, name=f"bw_{name}")
            _qs[idx % 2].dma_start(t[:], bigwd[128 * idx:128 * (idx + 1), :])
            big[name] = t
        cw2 = []
        for n in range(NA):
            t = wp.tile([128, 2], bf16, name=f"cw2_{n}")
            nc.sync.dma_start(t[:], cw2d[128 * n:128 * (n + 1), :])
            cw2.append(t)
        biast = wp.tile([128, 9], f32)
        nc.sync.dma_start(biast[:], biasd[:, :])
        bcol = {name: biast[:, i:i + 1] for i, name in enumerate(BIASC)}
        cb2t = wp.tile([2, NA], f32)
        nc.sync.dma_start(cb2t[:], cb2d[:, :])
        ident = wp.tile([128, 128], bf16)
        nc.scalar.dma_start(ident[:], identd[:, :])
        onesb = wp.tile([128, 1], bf16)
        nc.vector.memset(onesb[:], 1.0)
        zbias = wp.tile([128, 1], f32)
        nc.vector.memset(zbias[:], 0.0)


        # ---------- main loop: iteration PAIRS ----------
        # PSUM tags (8 banks):
        #  T0[2]: l1p0, v0p, kmpa, hp2_0   T1[2]: l1p1, v1pa, cvpa, hp2_1
        #  T2[2]: l1pG, skp, mp2, hp2_2    T3[1]: l1p3, v1pb, kmpb, cvpb, qp
        #  T5[1]: lgp, wfmP, clgp, cwfmP
        LR = mybir.ActivationFunctionType.Prelu
        EXP = mybir.ActivationFunctionType.Exp
        for ip in range(NPAIR):
            psl = slice(ip * 2 * NT, (ip + 1) * 2 * NT)
            se_t = {}
            enC_t = {}
            valsA = {}
            ov_t = {}

            def actor_a(n):
                """l1 + selectors + dots + softmax + weight store."""
                o = ENT_ROWS * n
                ebA = wk.tile([69, 2 * NT], bf16, name=f"ebA{n}", bufs=2)
                ebB = wk.tile([67, 2 * NT], bf16, name=f"ebB{n}", bufs=2)
                nc.gpsimd.dma_start(ebA[:], entd[o + 0:o + 69, psl])
                nc.gpsimd.dma_start(ebB[:], entd[o + 96:o + 163, psl])
                ebg = {"A": ebA, "B": ebB}
                lwg = {"A": lwA[n], "B": lwB[n], "C": lwC[n]}
                vals0_t = wk.tile([128, 2048], bf16, name="vals0", bufs=3)
                vals1_t = wk.tile([128, 3072], bf16, name="vals1", bufs=3)
                valsA[n] = (vals0_t, vals1_t)
                enC = wk.tile([128, 1024], bf16, name="enC", bufs=3)
                enC_t[n] = enC
                lgp = pp.tile([128, 64], f32, name="lgp", tag="T5")
                for h in range(2):
                    hsl = slice(h * NT, (h + 1) * NT)
                    sl = slice((2 * ip + h) * NT, (2 * ip + h + 1) * NT)
                    l1p0 = pp.tile([128, 1024], f32, name="l1p0", tag="T0")
                    l1p1 = pp.tile([128, 1024], f32, name="l1p1", tag="T1")
                    l1pG = pp.tile([128, 1024], f32, name="l1pG", tag="T2")
                    l1p3 = pp.tile([128, 512], f32, name="l1p3", tag="T3")
                    dests = {"en": (l1p0, 0), "oa0": (l1p0, 512),
                             "oa1": (l1p1, 0), "g0": (l1p1, 512),
                             "g1": (l1pG, 0), "g2": (l1pG, 512),
                             "senc": (l1p3, 0)}
                    for bname, g, base, K in BLOCKS:
                        pt, off = dests[bname]
                        rhs = (ebC[n][:, sl] if g == "C"
                               else ebg[g][base:base + K + 1, hsl])
                        nc.tensor.matmul(pt[:, off:off + NT],
                                         lwg[g][base:base + K + 1, :], rhs,
                                         start=True, stop=True)
                    l1x = wk.tile([128, 3072], bf16, name="l1x", bufs=2)
                    se = wk.tile([128, 512], bf16, name="se", bufs=6)
                    nc.scalar.activation(l1x[:, 0:1024], l1p0[:], LR,
                                         bias=zbias[:], alpha=0.01)
                    nc.scalar.activation(l1x[:, 1024:2048], l1p1[:], LR,
                                         bias=zbias[:], alpha=0.01)
                    nc.scalar.activation(l1x[:, 2048:3072], l1pG[:], LR,
                                         bias=zbias[:], alpha=0.01)
                    nc.scalar.activation(se[:], l1p3[:], LR,
                                         bias=zbias[:], alpha=0.01)
                    # second copy of the en encoding outliving l1x (merge
                    # input in phase B2)
                    nc.scalar.activation(enC[:, 512 * h:512 * (h + 1)],
                                         l1p0[:, 0:512], LR,
                                         bias=zbias[:], alpha=0.01)
                    se_t[(h, n)] = se
                    skp = pp.tile([128, 1024], f32, name="skp", tag="T2")
                    en_ = l1x[:, 0:512]
                    nc.tensor.matmul(skp[:, 0:512], big["wsk0"][:], en_,
                                     start=True, stop=True)
                    nc.tensor.matmul(skp[:, 512:1024], big["wsk1"][:], en_,
                                     start=True, stop=True)
                    # products on vector while the PE streams the vals
                    prs = []
                    for p in range(5):
                        sk = skp[:, 0:512] if p < 2 else skp[:, 512:1024]
                        enc = l1x[:, 512 * (p + 1):512 * (p + 2)]
                        pr = wk.tile([128, 512], bf16, name="pr", bufs=6)
                        nc.vector.tensor_tensor(out=pr[:], in0=sk, in1=enc,
                                                op=mybir.AluOpType.mult)
                        prs.append(pr)
                    v0p = pp.tile([128, 1024], f32, name="v0p", tag="T0")
                    nc.tensor.matmul(v0p[:, 0:512], big["aval0"][:],
                                     l1x[:, 512:1024], start=True, stop=True)
                    nc.tensor.matmul(v0p[:, 512:1024], big["aval0"][:],
                                     l1x[:, 1024:1536], start=True, stop=True)
                    nc.scalar.activation(vals0_t[:, 1024 * h:1024 * (h + 1)],
                                         v0p[:], LR, bias=bcol["avb0"],
                                         alpha=0.01)
                    v1pa = pp.tile([128, 1024], f32, name="v1pa", tag="T1")
                    v1pb = pp.tile([128, 512], f32, name="v1pb", tag="T3")
                    nc.tensor.matmul(v1pa[:, 0:512], big["aval1"][:],
                                     l1x[:, 1536:2048], start=True, stop=True)
                    nc.tensor.matmul(v1pa[:, 512:1024], big["aval1"][:],
                                     l1x[:, 2048:2560], start=True, stop=True)
                    nc.tensor.matmul(v1pb[:], big["aval1"][:],
                                     l1x[:, 2560:3072], start=True, stop=True)
                    nc.scalar.activation(vals1_t[:, 1536 * h:1536 * h + 1024],
                                         v1pa[:], LR, bias=bcol["avb1"],
                                         alpha=0.01)
                    nc.scalar.activation(
                        vals1_t[:, 1536 * h + 1024:1536 * h + 1536],
                        v1pb[:], LR, bias=bcol["avb1"], alpha=0.01)
                    # batch-major dots: column 8p + (4h + t)
                    for p in range(5):
                        for t in range(4):
                            col = 8 * p + 4 * h + t
                            nc.tensor.matmul(lgp[:, col:col + 1],
                                             prs[p][:, 128 * t:128 * (t + 1)],
                                             onesb[:], start=True, stop=True)
                # softmax over slots, batch-major (u = 4h+t is inner dim)
                ebm = wk.tile([128, 40], bf16, name="ebm")
                nc.scalar.activation(ebm[:], lgp[:, 0:40], EXP, scale=SCALE)
                den = wk.tile([128, 16], f32, name="den")
                nc.vector.tensor_reduce(
                    out=den[:].rearrange("q (u g) -> q u g", g=2)[:, :, 0:1],
                    in_=ebm[:].rearrange("q (p u) -> q u p", u=8)[:, :, 0:2],
                    op=mybir.AluOpType.add, axis=mybir.AxisListType.X)
                nc.vector.tensor_reduce(
                    out=den[:].rearrange("q (u g) -> q u g", g=2)[:, :, 1:2],
                    in_=ebm[:].rearrange("q (p u) -> q u p", u=8)[:, :, 2:5],
                    op=mybir.AluOpType.add, axis=mybir.AxisListType.X)
                rec = wk.tile([128, 16], f32, name="rec")
                nc.vector.reciprocal(rec[:], den[:])
                wbm = wk.tile([128, 64], bf16, name="wbm")
                nc.vector.tensor_tensor(
                    out=wbm[:, 0:40].rearrange("q (p u) -> q u p", u=8)
                    [:, :, 0:2],
                    in0=ebm[:].rearrange("q (p u) -> q u p", u=8)[:, :, 0:2],
                    in1=rec[:].rearrange("q (u g) -> q u g", g=2)[:, :, 0:1]
                    .broadcast_to((128, 8, 2)),
                    op=mybir.AluOpType.mult)
                nc.vector.tensor_tensor(
                    out=wbm[:, 0:40].rearrange("q (p u) -> q u p", u=8)
                    [:, :, 2:5],
                    in0=ebm[:].rearrange("q (p u) -> q u p", u=8)[:, :, 2:5],
                    in1=rec[:].rearrange("q (u g) -> q u g", g=2)[:, :, 1:2]
                    .broadcast_to((128, 8, 3)),
                    op=mybir.AluOpType.mult)
                nc.gpsimd.memset(wbm[:, 40:64], 0.0)
                wfmP = pp.tile([64, 128], bf16, name="wfmP", tag="T5")
                nc.tensor.transpose(wfmP[:], wbm[:, 0:64], ident[:])
                wfm = wk.tile([64, 128], bf16, name="wfm", bufs=2)
                nc.vector.tensor_copy(wfm[:], wfmP[:])
                nc.sync.dma_start(
                    wscrd[ip, n].rearrange("s (r b) -> (s r) b", b=128),
                    wfm[:])

            def actor_b1(n):
                """broadcast weights + weighted sums (no PE work)."""
                vals0_t, vals1_t = valsA[n]
                scs = []
                for p in range(5):
                    wb_ = wk.tile([128, 1024], bf16, name="wb", bufs=3)
                    _qeng = [nc.sync, nc.gpsimd, nc.sync, nc.gpsimd,
                             nc.sync][p]
                    _qeng.dma_start(
                        wb_[:],
                        wscrd[ip, n, p:p + 1, :].broadcast_to((128, 1024)))
                    sc = wk.tile([128, 1024], bf16, name="sc", bufs=3)
                    if p < 2:
                        vin = vals0_t[:, :].rearrange(
                            "p (h q b) -> p h q b", h=2, q=2)[:, :, p, :]
                    else:
                        vin = vals1_t[:, :].rearrange(
                            "p (h q b) -> p h q b", h=2, q=3)[:, :, p - 2, :]
                    seng = nc.gpsimd if p == 2 else nc.vector
                    seng.tensor_tensor(
                        out=sc[:].rearrange("p (h b) -> p h b", h=2),
                        in0=vin, in1=wb_[:].rearrange("p (h b) -> p h b", h=2),
                        op=mybir.AluOpType.mult)
                    scs.append(sc)
                ov0 = wk.tile([128, 1024], bf16, name="ov0", bufs=3)
                nc.vector.tensor_tensor(out=ov0[:], in0=scs[0][:],
                                        in1=scs[1][:], op=mybir.AluOpType.add)
                ov1t = wk.tile([128, 1024], bf16, name="ov1t", bufs=1)
                nc.gpsimd.tensor_tensor(out=ov1t[:], in0=scs[2][:],
                                        in1=scs[3][:], op=mybir.AluOpType.add)
                ov1 = wk.tile([128, 1024], bf16, name="ov1", bufs=3)
                nc.vector.tensor_tensor(out=ov1[:], in0=ov1t[:],
                                        in1=scs[4][:], op=mybir.AluOpType.add)
                ov_t[n] = (ov0, ov1)

            mp2_t = {}

            def actor_b2a(n):
                """independent m_en matmuls: PE filler while the
                broadcast-dependent weighted sums drain on V/DMA."""
                mp2 = pp.tile([128, 1024], f32, name=f"mp2_{n}",
                              tag=["T0", "T1", "T2"][n])
                mp2_t[n] = mp2
                for h in range(2):
                    nc.tensor.matmul(mp2[:, 512 * h:512 * (h + 1)],
                                     big[f"m_en{n}"][:],
                                     enC_t[n][:, 512 * h:512 * (h + 1)],
                                     start=True, stop=False)

            def actor_b2b(n):
                """ov merge matmuls + sa activation."""
                ov0, ov1 = ov_t[n]
                mp2 = mp2_t[n]
                for h in range(2):
                    nc.tensor.matmul(mp2[:, 512 * h:512 * (h + 1)],
                                     big[f"m_ov0{n}"][:],
                                     ov0[:, 512 * h:512 * (h + 1)],
                                     start=False, stop=False)
                    nc.tensor.matmul(mp2[:, 512 * h:512 * (h + 1)],
                                     big[f"m_ov1{n}"][:],
                                     ov1[:, 512 * h:512 * (h + 1)],
                                     start=False, stop=True)
                for h in range(2):
                    nc.scalar.activation(saH[h][:, 512 * n:512 * (n + 1)],
                                         mp2[:, 512 * h:512 * (h + 1)], LR,
                                         bias=bcol[f"mb{n}"], alpha=0.01)

            saH = {h: wk.tile([128, 1536], bf16, name=f"saH{h}", bufs=2)
                   for h in range(2)}
            for n in range(NA):
                actor_a(n)
            for n in range(NA):
                actor_b1(n)
            for n in range(NA):
                actor_b2a(n)
            for n in range(NA):
                actor_b2b(n)

            # ---- critic ----
            keysM = wk.tile([128, 3072], bf16, name="keysM", bufs=1)
            cval = wk.tile([128, 3072], bf16, name="cval", bufs=1)
            clgp = pp.tile([128, 64], f32, name="clgp", tag="T5")
            for h in range(2):
                kmpa = pp.tile([128, 1024], f32, name="kmpa", tag="T0")
                kmpb = pp.tile([128, 512], f32, name="kmpb", tag="T3")
                nc.tensor.matmul(kmpa[:, 0:512], big["mcrit"][:],
                                 saH[h][:, 0:512], start=True, stop=True)
                nc.tensor.matmul(kmpa[:, 512:1024], big["mcrit"][:],
                                 saH[h][:, 512:1024], start=True, stop=True)
                nc.tensor.matmul(kmpb[:], big["mcrit"][:],
                                 saH[h][:, 1024:1536], start=True, stop=True)
                nc.vector.tensor_copy(keysM[:, 1536 * h:1536 * h + 1024],
                                      kmpa[:])
                nc.vector.tensor_copy(
                    keysM[:, 1536 * h + 1024:1536 * h + 1536], kmpb[:])
                cvpa = pp.tile([128, 1024], f32, name="cvpa", tag="T1")
                cvpb = pp.tile([128, 512], f32, name="cvpb", tag="T3")
                nc.tensor.matmul(cvpa[:, 0:512], big["cvalw"][:],
                                 saH[h][:, 0:512], start=True, stop=True)
                nc.tensor.matmul(cvpa[:, 512:1024], big["cvalw"][:],
                                 saH[h][:, 512:1024], start=True, stop=True)
                nc.tensor.matmul(cvpb[:], big["cvalw"][:],
                                 saH[h][:, 1024:1536], start=True, stop=True)
                nc.scalar.activation(cval[:, 1536 * h:1536 * h + 1024],
                                     cvpa[:], LR, bias=bcol["cvb"],
                                     alpha=0.01)
                nc.scalar.activation(
                    cval[:, 1536 * h + 1024:1536 * h + 1536],
                    cvpb[:], LR, bias=bcol["cvb"], alpha=0.01)
                for i in range(NA):
                    js = [j for j in range(NA) if j != i]
                    for k, j in enumerate(js):
                        c = 2 * i + k
                        prc = wk.tile([128, 512], bf16, name="prc", bufs=6)
                        nc.vector.tensor_tensor(
                            out=prc[:], in0=se_t[(h, i)][:],
                            in1=keysM[:, 1536 * h + 512 * j:
                                      1536 * h + 512 * (j + 1)],
                            op=mybir.AluOpType.mult)
                        for t in range(4):
                            col = 8 * c + 4 * h + t
                            nc.tensor.matmul(
                                clgp[:, col:col + 1],
                                prc[:, 128 * t:128 * (t + 1)],
                                onesb[:], start=True, stop=True)
            cebm = wk.tile([128, 48], bf16, name="cebm")
            nc.scalar.activation(cebm[:], clgp[:, 0:48], EXP, scale=SCALE)
            cden = wk.tile([128, 24], f32, name="cden")
            nc.vector.tensor_reduce(
                out=cden[:].rearrange("q (u i) -> q u i", i=3)
                    .rearrange("q u i -> q u i ()"),
                in_=cebm[:].rearrange("q (i k u) -> q u i k", i=3, k=2),
                op=mybir.AluOpType.add, axis=mybir.AxisListType.X)
            crec = wk.tile([128, 24], f32, name="crec")
            nc.vector.reciprocal(crec[:], cden[:])
            cwbm = wk.tile([128, 64], bf16, name="cwbm")
            nc.vector.tensor_tensor(
                out=cwbm[:, 0:48].rearrange("q (i k u) -> q u i k",
                                            i=3, k=2),
                in0=cebm[:].rearrange("q (i k u) -> q u i k", i=3, k=2),
                in1=crec[:].rearrange("q (u i) -> q u i", i=3)
                .rearrange("q u i -> q u i ()").broadcast_to((128, 8, 3, 2)),
                op=mybir.AluOpType.mult)
            nc.gpsimd.memset(cwbm[:, 48:64], 0.0)
            cwfmP = pp.tile([64, 128], bf16, name="cwfmP", tag="T5")
            nc.tensor.transpose(cwfmP[:], cwbm[:, 0:64], ident[:])
            cwfm = wk.tile([64, 128], bf16, name="cwfm", bufs=2)
            nc.vector.tensor_copy(cwfm[:], cwfmP[:])
            nc.sync.dma_start(
                wscrd[ip, 3].rearrange("s (r b) -> (s r) b", b=128),
                cwfm[:])
            # phase C1 tail: independent cw1a matmuls cover the broadcast
            hp2s = {}
            for i in range(NA):
                hp2 = pp.tile([128, 1024], f32, name=f"hp2_{i}",
                              tag=["T2", "T1", "T0"][i])
                hp2s[i] = hp2
                for h in range(2):
                    nc.tensor.matmul(hp2[:, 512 * h:512 * (h + 1)],
                                     big[f"cw1a{i}"][:], se_t[(h, i)][:],
                                     start=True, stop=False)
            covs = {}
            for i in range(NA):
                cscs = []
                for k in range(2):
                    c = 2 * i + k
                    j = [j for j in range(NA) if j != i][k]
                    cwb = wk.tile([128, 1024], bf16, name="cwb", bufs=3)
                    _qeng = [nc.sync, nc.gpsimd][k]
                    _qeng.dma_start(
                        cwb[:],
                        wscrd[ip, 3, c:c + 1, :].broadcast_to((128, 1024)))
                    csc = wk.tile([128, 1024], bf16, name="csc", bufs=3)
                    ceng = nc.gpsimd if k == 0 else nc.vector
                    ceng.tensor_tensor(
                        out=csc[:].rearrange("p (h b) -> p h b", h=2),
                        in0=cval[:, :].rearrange("p (h q b) -> p h q b",
                                                 h=2, q=3)[:, :, j, :],
                        in1=cwb[:].rearrange("p (h b) -> p h b", h=2),
                        op=mybir.AluOpType.mult)
                    cscs.append(csc)
                cov = wk.tile([128, 1024], bf16, name="cov", bufs=3)
                nc.vector.tensor_tensor(out=cov[:], in0=cscs[0][:],
                                        in1=cscs[1][:],
                                        op=mybir.AluOpType.add)
                covs[i] = cov
            for i in range(NA):
                hp2 = hp2s[i]
                cov = covs[i]
                for h in range(2):
                    nc.tensor.matmul(hp2[:, 512 * h:512 * (h + 1)],
                                     big[f"cw1b{i}"][:],
                                     cov[:, 512 * h:512 * (h + 1)],
                                     start=False, stop=True)
                for h in range(2):
                    it = 2 * ip + h
                    sl = slice(it * NT, (it + 1) * NT)
                    h_ = wk.tile([128, 512], bf16, name="h", bufs=2)
                    nc.scalar.activation(h_[:],
                                         hp2[:, 512 * h:512 * (h + 1)], LR,
                                         bias=bcol[f"cb1{i}"], alpha=0.01)
                    qp = pp.tile([2, 512], f32, name="qp", tag="T3")
                    nc.tensor.matmul(qp[:], cw2[i][:], h_[:], start=True,
                                     stop=True)
                    qs = wk.tile([2, 512], f32, name="qs", bufs=2)
                    nc.scalar.activation(qs[:], qp[:],
                                         mybir.ActivationFunctionType.Identity,
                                         bias=cb2t[:, i:i + 1])
                    nc.sync.dma_start(outd[2 * i:2 * i + 2, sl], qs[:])

    nc.compile()
    return nc


def _get_nc():
    if "nc" not in _NC_CACHE:
        _NC_CACHE["nc"] = _build_nc()
    return _NC_CACHE["nc"]


def kernel(s, a, en_W, en_b, oa_W, oa_b, goal_W, goal_b, akey_W, asel_W,
           aval_W, aval_b, merge_W, merge_b, senc_W, senc_b, ckey_W,
           csel_W, cval_W, cval_b, cW1, cb1, cW2, cb2):
    inp = dict(s=s, a=a, en_W=en_W, en_b=en_b, oa_W=oa_W, oa_b=oa_b,
               goal_W=goal_W, goal_b=goal_b, akey_W=akey_W, asel_W=asel_W,
               aval_W=aval_W, aval_b=aval_b, merge_W=merge_W, merge_b=merge_b,
               senc_W=senc_W, senc_b=senc_b, ckey_W=ckey_W, csel_W=csel_W,
               cval_W=cval_W, cval_b=cval_b, cW1=cW1, cb1=cb1, cW2=cW2,
               cb2=cb2)
    inp = {k: np.asarray(v, np.float32) for k, v in inp.items()}
    s_, a_ = inp["s"], inp["a"]

    l1A, l1B, l1C = _prep_l1w(inp)
    bigw = _prep_bigw(inp)
    cw2 = _b16(np.concatenate([inp["cW2"][n] for n in range(NA)], 0))
    biasc = _prep_bias(inp)
    cb2c = inp["cb2"].T.copy()
    ident = _b16(np.eye(128, dtype=np.float32))

    in_maps = []
    for c in range(NCORES):
        ent = _prep_ent_blocks(s_, a_, c * BS, (c + 1) * BS)
        in_maps.append({"entd": ent, "l1Ad": l1A, "l1Bd": l1B, "l1Cd": l1C,
                        "bigwd": bigw, "cw2d": cw2, "biasd": biasc,
                        "cb2d": cb2c, "identd": ident})

    nc = _get_nc()
    trace = os.environ.get("BASS_KERNEL_TRACE") == "1"
    res = run_bass_kernel_spmd(nc, in_maps, core_ids=list(range(NCORES)),
                               trace=trace)
    if trace:
        kernel.last_exec_time_ns = res.exec_time_ns
        kernel.last_results = res

    qfull = np.concatenate([res.results[c]["outd"] for c in range(NCORES)], 1)
    return np.ascontiguousarray(
        np.transpose(qfull.reshape(NA, 2, B), (0, 2, 1))).astype(np.float32)


# revision 33
# speedup vs baseline: 1.1443x; 1.1443x over previous
"""Trainium2 Bass kernel for nn_Attention_Critic (gnn_message_passing).

Strategy v3: data-parallel over the batch (8 cores x 4096), feature-major
layout on chip ([features, batch]), BatchNorm folded into first-layer
weights ON HOST (stats over the full batch in numpy -- no device stats,
no AllReduce), attention-weight products folded on host (sel@key^T),
attention dots via cheap 1-column PE reduce matmuls into a batch-major
[128, 64] logit tile (slot-major columns 8p+u), softmax with tiny
batch-major vector ops, weights transposed to row-major via a single PE
transpose, stored to DRAM as contiguous [slots, 1024] rows and broadcast
back via stride-0 row reads.  The per-pair schedule is phased
(A: encode+dots+softmax for all agents; B1: broadcasts+weighted sums;
B2: merge matmuls) so the PE never sits behind the broadcast round trip
and no PSUM tag is held across another agent's encode.  bf16 matmuls,
fp32 PSUM.
"""
import os
import sys

sys.path.insert(0, "/opt/trn_rl_repo")

import numpy as np
import ml_dtypes
from contextlib import ExitStack

import concourse.bass as bass
import concourse.tile as tile
from concourse import bacc, mybir
from concourse.bass_utils import run_bass_kernel_spmd

# Pin every activation to the natural_log_exp_and_others table set (covers
# Exp/Prelu/Identity/Copy) so the whole kernel needs exactly one
# ACT_TABLE_LOAD instead of thrashing between per-function sets.
_ORIG_GAT = bacc.get_activation_tables


def _pinned_tables(arch):
    t = _ORIG_GAT(arch)
    return {k: (v if k == "natural_log_exp_and_others" else set())
            for k, v in t.items()}


bacc.get_activation_tables = _pinned_tables

NA, B, H = 3, 32768, 128
EPS = 1e-5
NCORES = 8
BS = B // NCORES          # 4096 per core
NT = 512                  # batch tile
ITERS = BS // NT          # 8
NPAIR = ITERS // 2        # 4 iteration pairs
SCALE = 1.0 / np.sqrt(H)

bf16 = mybir.dt.bfloat16
f32 = mybir.dt.float32

ENT_ROWS = 208            # per-agent stride in entd (96-aligned groups)
# (name, group, tile partition base, K)
BLOCKS = [("en", "A", 0, 6), ("oa0", "A", 32, 4), ("oa1", "A", 64, 4),
          ("g0", "B", 0, 2), ("g1", "B", 32, 2), ("g2", "B", 64, 2),
          ("senc", "C", 0, 18)]
BIGW = (["wsk0", "wsk1", "aval0", "aval1", "mcrit", "cvalw"]
        + [f"m_en{n}" for n in range(NA)] + [f"m_ov0{n}" for n in range(NA)]
        + [f"m_ov1{n}" for n in range(NA)] + [f"cw1a{n}" for n in range(NA)]
        + [f"cw1b{n}" for n in range(NA)])
BIASC = ["avb0", "avb1", "mb0", "mb1", "mb2", "cvb", "cb10", "cb11", "cb12"]


def _b16(x):
    return np.asarray(x, np.float32).astype(ml_dtypes.bfloat16)


def _prep_ent_blocks(s, a, lo, hi):
    bs = hi - lo
    out = np.zeros((NA * ENT_ROWS, bs), np.float32)
    for n in range(NA):
        sn = s[n, lo:hi].T
        an = a[n, lo:hi].T
        r = ENT_ROWS * n
        out[r + 0:r + 4] = sn[0:4]
        out[r + 4:r + 6] = an[0:2]
        out[r + 6] = 1.0
        out[r + 32:r + 36] = sn[4:8]
        out[r + 36] = 1.0
        out[r + 64:r + 68] = sn[8:12]
        out[r + 68] = 1.0
        out[r + 96:r + 98] = sn[12:14]
        out[r + 98] = 1.0
        out[r + 128:r + 130] = sn[14:16]
        out[r + 130] = 1.0
        out[r + 160:r + 162] = sn[16:18]
        out[r + 162] = 1.0
        out[r + 176:r + 194] = sn[0:18]
        out[r + 194] = 1.0
    return _b16(out)


def _fold(W, b, mean, std):
    """BatchNorm fold: y = ((x-m)/std) @ W + b  ->  x @ Wf + bf."""
    Wf = W / std[:, None]
    bf = b - (mean / std) @ W
    return Wf, bf


def _prep_l1w(inp):
    s, a = inp["s"], inp["a"]
    A = np.zeros((NA * 69, 128), np.float32)
    Bm = np.zeros((NA * 67, 128), np.float32)
    C = np.zeros((NA * 19, 128), np.float32)
    for n in range(NA):
        feats = np.concatenate([s[n], a[n][:, :2]], 1).astype(np.float64)
        mean = feats.mean(0).astype(np.float32)
        std = np.sqrt(feats.var(0) + EPS).astype(np.float32)

        def fold(W, b, idx):
            return _fold(W, b, mean[idx], std[idx])

        Wf, bf = fold(inp["en_W"][n], inp["en_b"][n], [0, 1, 2, 3, 18, 19])
        A[69 * n + 0:69 * n + 6] = Wf
        A[69 * n + 6] = bf
        Wf, bf = fold(inp["oa_W"][n], inp["oa_b"][n], [4, 5, 6, 7])
        A[69 * n + 32:69 * n + 36] = Wf
        A[69 * n + 36] = bf
        Wf, bf = fold(inp["oa_W"][n], inp["oa_b"][n], [8, 9, 10, 11])
        A[69 * n + 64:69 * n + 68] = Wf
        A[69 * n + 68] = bf
        for gi, base in enumerate((0, 32, 64)):
            idx = [12 + 2 * gi, 13 + 2 * gi]
            Wf, bf = fold(inp["goal_W"][n], inp["goal_b"][n], idx)
            Bm[67 * n + base:67 * n + base + 2] = Wf
            Bm[67 * n + base + 2] = bf
        Wf, bf = fold(inp["senc_W"][n], inp["senc_b"][n], list(range(18)))
        C[19 * n + 0:19 * n + 18] = Wf
        C[19 * n + 18] = bf
    return _b16(A), _b16(Bm), _b16(C)


def _prep_bigw(inp):
    w = {}
    w["wsk0"] = inp["asel_W"][0] @ inp["akey_W"][0].T
    w["wsk1"] = inp["asel_W"][1] @ inp["akey_W"][1].T
    w["aval0"] = inp["aval_W"][0]
    w["aval1"] = inp["aval_W"][1]
    w["mcrit"] = inp["ckey_W"][0] @ inp["csel_W"][0].T
    w["cvalw"] = inp["cval_W"][0]
    for n in range(NA):
        w[f"m_en{n}"] = inp["merge_W"][n, 0:128]
        w[f"m_ov0{n}"] = inp["merge_W"][n, 128:256]
        w[f"m_ov1{n}"] = inp["merge_W"][n, 256:384]
        w[f"cw1a{n}"] = inp["cW1"][n, 0:128]
        w[f"cw1b{n}"] = inp["cW1"][n, 128:256]
    return _b16(np.concatenate([w[k] for k in BIGW], 0))


def _prep_bias(inp):
    cols = [inp["aval_b"][0], inp["aval_b"][1],
            inp["merge_b"][0], inp["merge_b"][1], inp["merge_b"][2],
            inp["cval_b"][0], inp["cb1"][0], inp["cb1"][1], inp["cb1"][2]]
    return np.stack(cols, 1).astype(np.float32)


_NC_CACHE = {}


def _build_nc():
    nc = bacc.Bacc("TRN2", target_bir_lowering=False, debug=False,
                   num_devices=NCORES)
    entd = nc.dram_tensor("entd", [NA * ENT_ROWS, BS], bf16,
                          kind="ExternalInput")
    l1Ad = nc.dram_tensor("l1Ad", [NA * 69, 128], bf16, kind="ExternalInput")
    l1Bd = nc.dram_tensor("l1Bd", [NA * 67, 128], bf16, kind="ExternalInput")
    l1Cd = nc.dram_tensor("l1Cd", [NA * 19, 128], bf16, kind="ExternalInput")
    bigwd = nc.dram_tensor("bigwd", [21 * 128, 128], bf16,
                           kind="ExternalInput")
    cw2d = nc.dram_tensor("cw2d", [NA * 128, 2], bf16, kind="ExternalInput")
    biasd = nc.dram_tensor("biasd", [128, 9], f32, kind="ExternalInput")
    cb2d = nc.dram_tensor("cb2d", [2, NA], f32, kind="ExternalInput")
    identd = nc.dram_tensor("identd", [128, 128], bf16, kind="ExternalInput")
    outd = nc.dram_tensor("outd", [6, BS], f32, kind="ExternalOutput")

    wscrd = nc.dram_tensor("wscrd", [NPAIR, 4, 8, 1024], bf16)

    with tile.TileContext(nc) as tc, ExitStack() as ctx:
        wp = ctx.enter_context(tc.tile_pool(name="wp", bufs=1))
        io = ctx.enter_context(tc.tile_pool(name="io", bufs=1))
        wk = ctx.enter_context(tc.tile_pool(name="wk", bufs=2))
        pp = ctx.enter_context(tc.tile_pool(name="pp", bufs=1, space="PSUM"))

        lwA, lwB, lwC = {}, {}, {}
        for n in range(NA):
            tA = wp.tile([69, 128], bf16, name=f"lwA{n}")
            nc.scalar.dma_start(tA[:], l1Ad[69 * n:69 * (n + 1), :])
            lwA[n] = tA
            tB = wp.tile([67, 128], bf16, name=f"lwB{n}")
            nc.scalar.dma_start(tB[:], l1Bd[67 * n:67 * (n + 1), :])
            lwB[n] = tB
            tC = wp.tile([19, 128], bf16, name=f"lwC{n}")
            nc.scalar.dma_start(tC[:], l1Cd[19 * n:19 * (n + 1), :])
            lwC[n] = tC
        ebC = {}
        for n in range(NA):
            o = ENT_ROWS * n
            t = io.tile([19, BS], bf16, name=f"ebC{n}")
            nc.sync.dma_start(t[:], entd[o + 176:o + 195, :])
            ebC[n] = t
        big = {}
        _qs = [nc.sync, nc.scalar]
        for idx, name in enumerate(BIGW):
            t = wp.tile([128, 128], bf16, name=f"bw_{name}")
            _qs[idx % 2].dma_start(t[:], bigwd[128 * idx:128 * (idx + 1), :])
            big[name] = t
        cw2 = []
        for n in range(NA):
            t = wp.tile([128, 2], bf16, name=f"cw2_{n}")
            nc.sync.dma_start(t[:], cw2d[128 * n:128 * (n + 1), :])
            cw2.append(t)
        biast = wp.tile([128, 9], f32)
        nc.sync.dma_start(biast[:], biasd[:, :])
        bcol = {name: biast[:, i:i + 1] for i, name in enumerate(BIASC)}
        cb2t = wp.tile([2, NA], f32)
        nc.sync.dma_start(cb2t[:], cb2d[:, :])
        ident = wp.tile([128, 128], bf16)
        nc.scalar.dma_start(ident[:], identd[:, :])
        onesb = wp.tile([128, 1], bf16)
        nc.vector.memset(onesb[:], 1.0)
        zbias = wp.tile([128, 1], f32)
        nc.vector.memset(zbias[:], 0.0)


        # ---------- main loop: iteration PAIRS ----------
        # PSUM tags (8 banks):
        #  T0[2]: l1p0, v0p, kmpa, hp2_0   T1[2]: l1p1, v1pa, cvpa, hp2_1
        #  T2[2]: l1pG, skp, mp2, hp2_2    T3[1]: l1p3, v1pb, kmpb, cvpb, qp
        #  T5[1]: lgp, wfmP, clgp, cwfmP
        LR = mybir.ActivationFunctionType.Prelu
        EXP = mybir.ActivationFunctionType.Exp
        for ip in range(NPAIR):
            psl = slice(ip * 2 * NT, (ip + 1) * 2 * NT)
            se_t = {}
            enC_t = {}
            valsA = {}
            ov_t = {}

            def actor_a(n):
                """l1 + selectors + dots + softmax + weight store."""
                o = ENT_ROWS * n
                ebA = wk.tile([69, 2 * NT], bf16, name=f"ebA{n}", bufs=2)
                ebB = wk.tile([67, 2 * NT], bf16, name=f"ebB{n}", bufs=2)
                nc.gpsimd.dma_start(ebA[:], entd[o + 0:o + 69, psl])
                nc.gpsimd.dma_start(ebB[:], entd[o + 96:o + 163, psl])
                ebg = {"A": ebA, "B": ebB}
                lwg = {"A": lwA[n], "B": lwB[n], "C": lwC[n]}
                vals0_t = wk.tile([128, 2048], bf16, name="vals0", bufs=3)
                vals1_t = wk.tile([128, 3072], bf16, name="vals1", bufs=3)
                valsA[n] = (vals0_t, vals1_t)
                enC = wk.tile([128, 1024], bf16, name="enC", bufs=3)
                enC_t[n] = enC
                lgp = pp.tile([128, 64], f32, name="lgp", tag="T5")
                for h in range(2):
                    hsl = slice(h * NT, (h + 1) * NT)
                    sl = slice((2 * ip + h) * NT, (2 * ip + h + 1) * NT)
                    l1p0 = pp.tile([128, 1024], f32, name="l1p0", tag="T0")
                    l1p1 = pp.tile([128, 1024], f32, name="l1p1", tag="T1")
                    l1pG = pp.tile([128, 1024], f32, name="l1pG", tag="T2")
                    l1p3 = pp.tile([128, 512], f32, name="l1p3", tag="T3")
                    dests = {"en": (l1p0, 0), "oa0": (l1p0, 512),
                             "oa1": (l1p1, 0), "g0": (l1p1, 512),
                             "g1": (l1pG, 0), "g2": (l1pG, 512),
                             "senc": (l1p3, 0)}
                    for bname, g, base, K in BLOCKS:
                        pt, off = dests[bname]
                        rhs = (ebC[n][:, sl] if g == "C"
                               else ebg[g][base:base + K + 1, hsl])
                        nc.tensor.matmul(pt[:, off:off + NT],
                                         lwg[g][base:base + K + 1, :], rhs,
                                         start=True, stop=True)
                    l1x = wk.tile([128, 3072], bf16, name="l1x", bufs=2)
                    se = wk.tile([128, 512], bf16, name="se", bufs=6)
                    nc.scalar.activation(l1x[:, 0:1024], l1p0[:], LR,
                                         bias=zbias[:], alpha=0.01)
                    nc.scalar.activation(l1x[:, 1024:2048], l1p1[:], LR,
                                         bias=zbias[:], alpha=0.01)
                    nc.scalar.activation(l1x[:, 2048:3072], l1pG[:], LR,
                                         bias=zbias[:], alpha=0.01)
                    nc.scalar.activation(se[:], l1p3[:], LR,
                                         bias=zbias[:], alpha=0.01)
                    # second copy of the en encoding outliving l1x (merge
                    # input in phase B2)
                    nc.scalar.activation(enC[:, 512 * h:512 * (h + 1)],
                                         l1p0[:, 0:512], LR,
                                         bias=zbias[:], alpha=0.01)
                    se_t[(h, n)] = se
                    skp = pp.tile([128, 1024], f32, name="skp", tag="T2")
                    en_ = l1x[:, 0:512]
                    nc.tensor.matmul(skp[:, 0:512], big["wsk0"][:], en_,
                                     start=True, stop=True)
                    nc.tensor.matmul(skp[:, 512:1024], big["wsk1"][:], en_,
                                     start=True, stop=True)
                    # products on vector while the PE streams the vals
                    prs = []
                    for p in range(5):
                        sk = skp[:, 0:512] if p < 2 else skp[:, 512:1024]
                        enc = l1x[:, 512 * (p + 1):512 * (p + 2)]
                        pr = wk.tile([128, 512], bf16, name="pr", bufs=6)
                        nc.vector.tensor_tensor(out=pr[:], in0=sk, in1=enc,
                                                op=mybir.AluOpType.mult)
                        prs.append(pr)
                    v0p = pp.tile([128, 1024], f32, name="v0p", tag="T0")
                    nc.tensor.matmul(v0p[:, 0:512], big["aval0"][:],
                                     l1x[:, 512:1024], start=True, stop=True)
                    nc.tensor.matmul(v0p[:, 512:1024], big["aval0"][:],
                                     l1x[:, 1024:1536], start=True, stop=True)
                    nc.scalar.activation(vals0_t[:, 1024 * h:1024 * (h + 1)],
                                         v0p[:], LR, bias=bcol["avb0"],
                                         alpha=0.01)
                    v1pa = pp.tile([128, 1024], f32, name="v1pa", tag="T1")
                    v1pb = pp.tile([128, 512], f32, name="v1pb", tag="T3")
                    nc.tensor.matmul(v1pa[:, 0:512], big["aval1"][:],
                                     l1x[:, 1536:2048], start=True, stop=True)
                    nc.tensor.matmul(v1pa[:, 512:1024], big["aval1"][:],
                                     l1x[:, 2048:2560], start=True, stop=True)
                    nc.tensor.matmul(v1pb[:], big["aval1"][:],
                                     l1x[:, 2560:3072], start=True, stop=True)
                    nc.scalar.activation(vals1_t[:, 1536 * h:1536 * h + 1024],
                                         v1pa[:], LR, bias=bcol["avb1"],
                                         alpha=0.01)
                    nc.scalar.activation(
                        vals1_t[:, 1536 * h + 1024:1536 * h + 1536],
                        v1pb[:], LR, bias=bcol["avb1"], alpha=0.01)
                    # batch-major dots: column 8p + (4h + t)
                    for p in range(5):
                        for t in range(4):
                            col = 8 * p + 4 * h + t
                            nc.tensor.matmul(lgp[:, col:col + 1],
                                             prs[p][:, 128 * t:128 * (t + 1)],
                                             onesb[:], start=True, stop=True)
                # softmax over slots, batch-major (u = 4h+t is inner dim)
                ebm = wk.tile([128, 40], bf16, name="ebm")
                nc.scalar.activation(ebm[:], lgp[:, 0:40], EXP, scale=SCALE)
                den = wk.tile([128, 16], f32, name="den")
                nc.vector.tensor_reduce(
                    out=den[:].rearrange("q (u g) -> q u g", g=2)[:, :, 0:1],
                    in_=ebm[:].rearrange("q (p u) -> q u p", u=8)[:, :, 0:2],
                    op=mybir.AluOpType.add, axis=mybir.AxisListType.X)
                nc.vector.tensor_reduce(
                    out=den[:].rearrange("q (u g) -> q u g", g=2)[:, :, 1:2],
                    in_=ebm[:].rearrange("q (p u) -> q u p", u=8)[:, :, 2:5],
                    op=mybir.AluOpType.add, axis=mybir.AxisListType.X)
                rec = wk.tile([128, 16], f32, name="rec")
                nc.vector.reciprocal(rec[:], den[:])
                wbm = wk.tile([128, 64], bf16, name="wbm")
                nc.vector.tensor_tensor(
                    out=wbm[:, 0:40].rearrange("q (p u) -> q u p", u=8)
                    [:, :, 0:2],
                    in0=ebm[:].rearrange("q (p u) -> q u p", u=8)[:, :, 0:2],
                    in1=rec[:].rearrange("q (u g) -> q u g", g=2)[:, :, 0:1]
                    .broadcast_to((128, 8, 2)),
                    op=mybir.AluOpType.mult)
                nc.vector.tensor_tensor(
                    out=wbm[:, 0:40].rearrange("q (p u) -> q u p", u=8)
                    [:, :, 2:5],
                    in0=ebm[:].rearrange("q (p u) -> q u p", u=8)[:, :, 2:5],
                    in1=rec[:].rearrange("q (u g) -> q u g", g=2)[:, :, 1:2]
                    .broadcast_to((128, 8, 3)),
                    op=mybir.AluOpType.mult)
                nc.gpsimd.memset(wbm[:, 40:64], 0.0)
                wfmP = pp.tile([64, 128], bf16, name="wfmP", tag="T5")
                nc.tensor.transpose(wfmP[:], wbm[:, 0:64], ident[:])
                wfm = wk.tile([64, 128], bf16, name="wfm", bufs=2)
                nc.vector.tensor_copy(wfm[:], wfmP[:])
                nc.sync.dma_start(
                    wscrd[ip, n].rearrange("s (r b) -> (s r) b", b=128),
                    wfm[:])

            def actor_b1(n):
                """broadcast weights + weighted sums (no PE work)."""
                vals0_t, vals1_t = valsA[n]
                scs = []
                for p in range(5):
                    wb_ = wk.tile([128, 1024], bf16, name="wb", bufs=3)
                    _qeng = [nc.sync, nc.gpsimd, nc.sync, nc.gpsimd,
                             nc.sync][p]
                    _qeng.dma_start(
                        wb_[:],
                        wscrd[ip, n, p:p + 1, :].broadcast_to((128, 1024)))
                    sc = wk.tile([128, 1024], bf16, name="sc", bufs=3)
                    if p < 2:
                        vin = vals0_t[:, :].rearrange(
                            "p (h q b) -> p h q b", h=2, q=2)[:, :, p, :]
                    else:
                        vin = vals1_t[:, :].rearrange(
                            "p (h q b) -> p h q b", h=2, q=3)[:, :, p - 2, :]
                    seng = nc.gpsimd if p == 2 else nc.vector
                    seng.tensor_tensor(
                        out=sc[:].rearrange("p (h b) -> p h b", h=2),
                        in0=vin, in1=wb_[:].rearrange("p (h b) -> p h b", h=2),
                        op=mybir.AluOpType.mult)
                    scs.append(sc)
                ov0 = wk.tile([128, 1024], bf16, name="ov0", bufs=3)
                nc.vector.tensor_tensor(out=ov0[:], in0=scs[0][:],
                                        in1=scs[1][:], op=mybir.AluOpType.add)
                ov1t = wk.tile([128, 1024], bf16, name="ov1t", bufs=1)
                nc.gpsimd.tensor_tensor(out=ov1t[:], in0=scs[2][:],
                                        in1=scs[3][:], op=mybir.AluOpType.add)
                ov1 = wk.tile([128, 1024], bf16, name="ov1", bufs=3)
                nc.vector.tensor_tensor(out=ov1[:], in0=ov1t[:],
                                        in1=scs[4][:], op=mybir.AluOpType.add)
                ov_t[n] = (ov0, ov1)

            def actor_b2(n):
                """merge matmuls + sa activation (PE + ACT)."""
                ov0, ov1 = ov_t[n]
                mp2 = pp.tile([128, 1024], f32, name="mp2", tag="T2")
                for h in range(2):
                    nc.tensor.matmul(mp2[:, 512 * h:512 * (h + 1)],
                                     big[f"m_en{n}"][:],
                                     enC_t[n][:, 512 * h:512 * (h + 1)],
                                     start=True, stop=False)
                    nc.tensor.matmul(mp2[:, 512 * h:512 * (h + 1)],
                                     big[f"m_ov0{n}"][:],
                                     ov0[:, 512 * h:512 * (h + 1)],
                                     start=False, stop=False)
                    nc.tensor.matmul(mp2[:, 512 * h:512 * (h + 1)],
                                     big[f"m_ov1{n}"][:],
                                     ov1[:, 512 * h:512 * (h + 1)],
                                     start=False, stop=True)
                for h in range(2):
                    nc.scalar.activation(saH[h][:, 512 * n:512 * (n + 1)],
                                         mp2[:, 512 * h:512 * (h + 1)], LR,
                                         bias=bcol[f"mb{n}"], alpha=0.01)

            saH = {h: wk.tile([128, 1536], bf16, name=f"saH{h}", bufs=2)
                   for h in range(2)}
            for n in range(NA):
                actor_a(n)
            for n in range(NA):
                actor_b1(n)
            for n in range(NA):
                actor_b2(n)

            # ---- critic ----
            keysM = wk.tile([128, 3072], bf16, name="keysM", bufs=1)
            cval = wk.tile([128, 3072], bf16, name="cval", bufs=1)
            clgp = pp.tile([128, 64], f32, name="clgp", tag="T5")
            for h in range(2):
                kmpa = pp.tile([128, 1024], f32, name="kmpa", tag="T0")
                kmpb = pp.tile([128, 512], f32, name="kmpb", tag="T3")
                nc.tensor.matmul(kmpa[:, 0:512], big["mcrit"][:],
                                 saH[h][:, 0:512], start=True, stop=True)
                nc.tensor.matmul(kmpa[:, 512:1024], big["mcrit"][:],
                                 saH[h][:, 512:1024], start=True, stop=True)
                nc.tensor.matmul(kmpb[:], big["mcrit"][:],
                                 saH[h][:, 1024:1536], start=True, stop=True)
                nc.vector.tensor_copy(keysM[:, 1536 * h:1536 * h + 1024],
                                      kmpa[:])
                nc.vector.tensor_copy(
                    keysM[:, 1536 * h + 1024:1536 * h + 1536], kmpb[:])
                cvpa = pp.tile([128, 1024], f32, name="cvpa", tag="T1")
                cvpb = pp.tile([128, 512], f32, name="cvpb", tag="T3")
                nc.tensor.matmul(cvpa[:, 0:512], big["cvalw"][:],
                                 saH[h][:, 0:512], start=True, stop=True)
                nc.tensor.matmul(cvpa[:, 512:1024], big["cvalw"][:],
                                 saH[h][:, 512:1024], start=True, stop=True)
                nc.tensor.matmul(cvpb[:], big["cvalw"][:],
                                 saH[h][:, 1024:1536], start=True, stop=True)
                nc.scalar.activation(cval[:, 1536 * h:1536 * h + 1024],
                                     cvpa[:], LR, bias=bcol["cvb"],
                                     alpha=0.01)
                nc.scalar.activation(
                    cval[:, 1536 * h + 1024:1536 * h + 1536],
                    cvpb[:], LR, bias=bcol["cvb"], alpha=0.01)
                for i in range(NA):
                    js = [j for j in range(NA) if j != i]
                    for k, j in enumerate(js):
                        c = 2 * i + k
                        prc = wk.tile([128, 512], bf16, name="prc", bufs=6)
                        nc.vector.tensor_tensor(
                            out=prc[:], in0=se_t[(h, i)][:],
                            in1=keysM[:, 1536 * h + 512 * j:
                                      1536 * h + 512 * (j + 1)],
                            op=mybir.AluOpType.mult)
                        for t in range(4):
                            col = 8 * c + 4 * h + t
                            nc.tensor.matmul(
                                clgp[:, col:col + 1],
                                prc[:, 128 * t:128 * (t + 1)],
                                onesb[:], start=True, stop=True)
            cebm = wk.tile([128, 48], bf16, name="cebm")
            nc.scalar.activation(cebm[:], clgp[:, 0:48], EXP, scale=SCALE)
            cden = wk.tile([128, 24], f32, name="cden")
            nc.vector.tensor_reduce(
                out=cden[:].rearrange("q (u i) -> q u i", i=3)
                    .rearrange("q u i -> q u i ()"),
                in_=cebm[:].rearrange("q (i k u) -> q u i k", i=3, k=2),
                op=mybir.AluOpType.add, axis=mybir.AxisListType.X)
            crec = wk.tile([128, 24], f32, name="crec")
            nc.vector.reciprocal(crec[:], cden[:])
            cwbm = wk.tile([128, 64], bf16, name="cwbm")
            nc.vector.tensor_tensor(
                out=cwbm[:, 0:48].rearrange("q (i k u) -> q u i k",
                                            i=3, k=2),
                in0=cebm[:].rearrange("q (i k u) -> q u i k", i=3, k=2),
                in1=crec[:].rearrange("q (u i) -> q u i", i=3)
                .rearrange("q u i -> q u i ()").broadcast_to((128, 8, 3, 2)),
                op=mybir.AluOpType.mult)
            nc.gpsimd.memset(cwbm[:, 48:64], 0.0)
            cwfmP = pp.tile([64, 128], bf16, name="cwfmP", tag="T5")
            nc.tensor.transpose(cwfmP[:], cwbm[:, 0:64], ident[:])
            cwfm = wk.tile([64, 128], bf16, name="cwfm", bufs=2)
            nc.vector.tensor_copy(cwfm[:], cwfmP[:])
            nc.sync.dma_start(
                wscrd[ip, 3].rearrange("s (r b) -> (s r) b", b=128),
                cwfm[:])
            # phase C1 tail: independent cw1a matmuls cover the broadcast
            hp2s = {}
            for i in range(NA):
                hp2 = pp.tile([128, 1024], f32, name=f"hp2_{i}",
                              tag=["T2", "T1", "T0"][i])
                hp2s[i] = hp2
                for h in range(2):
                    nc.tensor.matmul(hp2[:, 512 * h:512 * (h + 1)],
                                     big[f"cw1a{i}"][:], se_t[(h, i)][:],
                                     start=True, stop=False)
            covs = {}
            for i in range(NA):
                cscs = []
                for k in range(2):
                    c = 2 * i + k
                    j = [j for j in range(NA) if j != i][k]
                    cwb = wk.tile([128, 1024], bf16, name="cwb", bufs=3)
                    _qeng = [nc.sync, nc.gpsimd][k]
                    _qeng.dma_start(
                        cwb[:],
                        wscrd[ip, 3, c:c + 1, :].broadcast_to((128, 1024)))
                    csc = wk.tile([128, 1024], bf16, name="csc", bufs=3)
                    ceng = nc.gpsimd if k == 0 else nc.vector
                    ceng.tensor_tensor(
                        out=csc[:].rearrange("p (h b) -> p h b", h=2),
                        in0=cval[:, :].rearrange("p (h q b) -> p h q b",
                                                 h=2, q=3)[:, :, j, :],
                        in1=cwb[:].rearrange("p (h b) -> p h b", h=2),
                        op=mybir.AluOpType.mult)
                    cscs.append(csc)
                cov = wk.tile([128, 1024], bf16, name="cov", bufs=3)
                nc.vector.tensor_tensor(out=cov[:], in0=cscs[0][:],
                                        in1=cscs[1][:],
                                        op=mybir.AluOpType.add)
                covs[i] = cov
            for i in range(NA):
                hp2 = hp2s[i]
                cov = covs[i]
                for h in range(2):
                    nc.tensor.matmul(hp2[:, 512 * h:512 * (h + 1)],
                                     big[f"cw1b{i}"][:],
                                     cov[:, 512 * h:512 * (h + 1)],
                                     start=False, stop=True)
                for h in range(2):
                    it = 2 * ip + h
                    sl = slice(it * NT, (it + 1) * NT)
                    h_ = wk.tile([128, 512], bf16, name="h", bufs=2)
                    nc.scalar.activation(h_[:],
                                         hp2[:, 512 * h:512 * (h + 1)], LR,
                                         bias=bcol[f"cb1{i}"], alpha=0.01)
                    qp = pp.tile([2, 512], f32, name="qp", tag="T3")
                    nc.tensor.matmul(qp[:], cw2[i][:], h_[:], start=True,
                                     stop=True)
                    qs = wk.tile([2, 512], f32, name="qs", bufs=2)
                    nc.scalar.activation(qs[:], qp[:],
                                         mybir.ActivationFunctionType.Identity,
                                         bias=cb2t[:, i:i + 1])
                    nc.sync.dma_start(outd[2 * i:2 * i + 2, sl], qs[:])

    nc.compile()
    return nc


def _get_nc():
    if "nc" not in _NC_CACHE:
        _NC_CACHE["nc"] = _build_nc()
    return _NC_CACHE["nc"]


def kernel(s, a, en_W, en_b, oa_W, oa_b, goal_W, goal_b, akey_W, asel_W,
           aval_W, aval_b, merge_W, merge_b, senc_W, senc_b, ckey_W,
           csel_W, cval_W, cval_b, cW1, cb1, cW2, cb2):
    inp = dict(s=s, a=a, en_W=en_W, en_b=en_b, oa_W=oa_W, oa_b=oa_b,
               goal_W=goal_W, goal_b=goal_b, akey_W=akey_W, asel_W=asel_W,
               aval_W=aval_W, aval_b=aval_b, merge_W=merge_W, merge_b=merge_b,
               senc_W=senc_W, senc_b=senc_b, ckey_W=ckey_W, csel_W=csel_W,
               cval_W=cval_W, cval_b=cval_b, cW1=cW1, cb1=cb1, cW2=cW2,
               cb2=cb2)
    inp = {k: np.asarray(v, np.float32) for k, v in inp.items()}
    s_, a_ = inp["s"], inp["a"]

    l1A, l1B, l1C = _prep_l1w(inp)
    bigw = _prep_bigw(inp)
    cw2 = _b16(np.concatenate([inp["cW2"][n] for n in range(NA)], 0))
    biasc = _prep_bias(inp)
    cb2c = inp["cb2"].T.copy()
    ident = _b16(np.eye(128, dtype=np.float32))

    in_maps = []
    for c in range(NCORES):
        ent = _prep_ent_blocks(s_, a_, c * BS, (c + 1) * BS)
        in_maps.append({"entd": ent, "l1Ad": l1A, "l1Bd": l1B, "l1Cd": l1C,
                        "bigwd": bigw, "cw2d": cw2, "biasd": biasc,
                        "cb2d": cb2c, "identd": ident})

    nc = _get_nc()
    trace = os.environ.get("BASS_KERNEL_TRACE") == "1"
    res = run_bass_kernel_spmd(nc, in_maps, core_ids=list(range(NCORES)),
                               trace=trace)
    if trace:
        kernel.last_exec_time_ns = res.exec_time_ns
        kernel.last_results = res

    qfull = np.concatenate([res.results[c]["outd"] for c in range(NCORES)], 1)
    return np.ascontiguousarray(
        np.transpose(qfull.reshape(NA, 2, B), (0, 2, 1))).astype(np.float32)
